# revision 2
# baseline (speedup 1.0000x reference)
"""Distributed Trainium2 kernel for nn_CONNECT_86964497809993 (TGN-style
GNN message passing: last-event aggregation + GRU memory update + community
incidence matmul), sharded over 8 NeuronCores by node id.

Strategy (per sharding hint): nodes are block-sharded across the 8 cores.
Event routing ("last message per node" selection) is pure integer index
plumbing, done on the host during input sharding; each core then runs the
full FP pipeline for its 12 500 nodes on-device:
  - time encoding  tenc = cos(dt*w + b) via range-reduced Sin LUT
  - gates          gx + gh = [embs|embd|feat|tenc|1] @ [W_ih;bias] + mem @ W_hh
  - GRU            r,z = sigmoid;  n = tanh(xn + r*hn);  h' = (1-z)n + z*mem
  - mask           new_mem = has ? h' : mem
  - community      commT += new_mem_tile.T @ inc_tile  (psum-accumulated)
GRU/time-encoding params are replicated to all cores; the [C,M] community
matmul partials are summed on the host (8 tiny [128,256] tiles).

Matmuls run as float32r (fp32 transpose-mode weight load, ~4x faster than
plain fp32 on the PE, ~1.5e-4 rel err)."""

import numpy as np

from concourse import bacc
import concourse.mybir as mybir
from concourse.tile import TileContext
from concourse.bass_utils import run_bass_kernel_spmd

# Problem shapes (hardcoded per contract).
N, E, C = 100000, 50000, 256
M, D, F, T = 128, 128, 128, 64
NCORES = 8
NPC = N // NCORES          # 12500 nodes per core
SLAB = 512                 # nodes per pipeline slab
P = 128
NTILES = (NPC + P - 1) // P  # 98 node-tiles per core

f32 = mybir.dt.float32
f32r = mybir.dt.float32r
A = mybir.AluOpType
AF = mybir.ActivationFunctionType

_COMPILED = None  # (nc,) cache — build/compile once per process


def _build_program():
    nc = bacc.Bacc("TRN2", target_bir_lowering=False)

    # Per-core inputs. Feature-major (transposed) copies feed the PE as lhsT;
    # node-major mem/inc feed elementwise ops and the community matmul.
    sT = nc.dram_tensor("sT", [D, NPC], f32r, kind="ExternalInput")
    dT = nc.dram_tensor("dT", [D, NPC], f32r, kind="ExternalInput")
    fT = nc.dram_tensor("fT", [F, NPC], f32r, kind="ExternalInput")
    mT = nc.dram_tensor("mT", [M, NPC], f32r, kind="ExternalInput")
    yT = nc.dram_tensor("yT", [T + 1, NPC], f32, kind="ExternalInput")  # sin args; row T = pi/2 (bias row -> 1.0)
    mem = nc.dram_tensor("mem", [NPC, M], f32, kind="ExternalInput")
    inc = nc.dram_tensor("inc", [NPC, C], f32r, kind="ExternalInput")
    hasp = nc.dram_tensor("hasp", [P, NTILES], f32, kind="ExternalInput")
    omhp = nc.dram_tensor("omhp", [P, NTILES], f32, kind="ExternalInput")
    Wa = nc.dram_tensor("Wa", [D, 3 * M], f32r, kind="ExternalInput")
    Wb = nc.dram_tensor("Wb", [D, 3 * M], f32r, kind="ExternalInput")
    Wc = nc.dram_tensor("Wc", [F, 3 * M], f32r, kind="ExternalInput")
    Wt = nc.dram_tensor("Wt", [T + 1, 3 * M], f32r, kind="ExternalInput")  # row T = b_ih + b_hh
    Whh = nc.dram_tensor("Whh", [M, 3 * M], f32r, kind="ExternalInput")

    om = nc.dram_tensor("om", [NPC, M], f32, kind="ExternalOutput")
    ocm = nc.dram_tensor("ocm", [M, C], f32, kind="ExternalOutput")

    with TileContext(nc) as tc:
        with tc.tile_pool(name="const", bufs=1) as cpool, \
             tc.tile_pool(name="sbuf", bufs=3) as pool, \
             tc.tile_pool(name="ps", bufs=3, space="PSUM") as pspool, \
             tc.tile_pool(name="pacc", bufs=1, space="PSUM") as paccpool:

            # Persistent tiles
            wa_t = cpool.tile([D, 3 * M], f32r)
            nc.sync.dma_start(wa_t[:], Wa[:])
            wb_t = cpool.tile([D, 3 * M], f32r)
            nc.sync.dma_start(wb_t[:], Wb[:])
            wc_t = cpool.tile([F, 3 * M], f32r)
            nc.sync.dma_start(wc_t[:], Wc[:])
            wt_t = cpool.tile([T + 1, 3 * M], f32r)
            nc.sync.dma_start(wt_t[:], Wt[:])
            whh_t = cpool.tile([M, 3 * M], f32r)
            nc.sync.dma_start(whh_t[:], Whh[:])
            has_t = cpool.tile([P, NTILES], f32)
            nc.sync.dma_start(has_t[:], hasp[:])
            omh_t = cpool.tile([P, NTILES], f32)
            nc.sync.dma_start(omh_t[:], omhp[:])

            comm_acc = paccpool.tile([M, C], f32)

            tile_idx = 0
            for g0 in range(0, NPC, SLAB):
                w = min(SLAB, NPC - g0)
                subs = []
                off = 0
                while off < w:
                    subs.append((off, min(P, w - off)))
                    off += subs[-1][1]
                nsub = len(subs)

                sT_s = pool.tile([D, w], f32r, tag="sT")
                nc.sync.dma_start(sT_s[:], sT[:, g0:g0 + w])
                dT_s = pool.tile([D, w], f32r, tag="dT")
                nc.sync.dma_start(dT_s[:], dT[:, g0:g0 + w])
                fT_s = pool.tile([F, w], f32r, tag="fT")
                nc.sync.dma_start(fT_s[:], fT[:, g0:g0 + w])
                mT_s = pool.tile([M, w], f32r, tag="mT")
                nc.sync.dma_start(mT_s[:], mT[:, g0:g0 + w])
                yT_s = pool.tile([T + 1, w], f32, tag="yT")
                nc.sync.dma_start(yT_s[:], yT[:, g0:g0 + w])

                mem_s = pool.tile([P, nsub, M], f32, tag="mem")
                inc_s = pool.tile([P, nsub, C], f32r, tag="inc")
                if w == SLAB:
                    nc.sync.dma_start(
                        mem_s[:], mem[g0:g0 + w].rearrange("(s p) f -> p s f", p=P))
                    nc.sync.dma_start(
                        inc_s[:], inc[g0:g0 + w].rearrange("(s p) f -> p s f", p=P))
                else:
                    for s, (soff, sw) in enumerate(subs):
                        nc.sync.dma_start(mem_s[:sw, s, :], mem[g0 + soff:g0 + soff + sw])
                        nc.sync.dma_start(inc_s[:sw, s, :], inc[g0 + soff:g0 + soff + sw])

                # tenc = sin(y') ; row T is pi/2 -> 1.0 bias lane
                tencf = pool.tile([T + 1, w], f32, tag="tencf")
                nc.scalar.activation(tencf[:], yT_s[:], AF.Sin)
                tencr = pool.tile([T + 1, w], f32r, tag="tencr")
                nc.vector.tensor_copy(tencr[:], tencf[:])

                n_sl = pool.tile([P, nsub, M], f32, tag="n_sl")
                zp_sl = pool.tile([P, nsub, M], f32, tag="zp_sl")

                for s, (soff, sw) in enumerate(subs):
                    gt = tile_idx
                    pall = pspool.tile([P, 3 * M], f32, tag="pall")
                    phn = pspool.tile([P, M], f32, tag="phn")
                    sl = slice(soff, soff + sw)
                    nc.tensor.matmul(pall[:sw, :], sT_s[:, sl], wa_t[:], start=True, stop=False)
                    nc.tensor.matmul(pall[:sw, :], dT_s[:, sl], wb_t[:], start=False, stop=False)
                    nc.tensor.matmul(pall[:sw, :], fT_s[:, sl], wc_t[:], start=False, stop=False)
                    nc.tensor.matmul(pall[:sw, :], tencr[:, sl], wt_t[:], start=False, stop=False)
                    nc.tensor.matmul(pall[:sw, 0:2 * M], mT_s[:, sl], whh_t[:, 0:2 * M],
                                     start=False, stop=True)
                    nc.tensor.matmul(phn[:sw, :], mT_s[:, sl], whh_t[:, 2 * M:3 * M],
                                     start=True, stop=True)

                    # r,z = sigmoid(gx+gh);  n = tanh(xn + r*hn)
                    rz = pool.tile([P, 2 * M], f32, tag="rz")
                    nc.scalar.activation(rz[:sw, :], pall[:sw, 0:2 * M], AF.Sigmoid)
                    tt = pool.tile([P, M], f32, tag="tt")
                    nc.vector.tensor_tensor(tt[:sw, :], rz[:sw, 0:M], phn[:sw, :], A.mult)
                    npre = pool.tile([P, M], f32, tag="npre")
                    nc.vector.tensor_tensor(npre[:sw, :], pall[:sw, 2 * M:3 * M], tt[:sw, :],
                                            A.add)
                    nc.scalar.activation(n_sl[:sw, s, :], npre[:sw, :], AF.Tanh)
                    # z' = has*z + (1-has):   empty nodes behave as z=1 (keep mem)
                    nc.vector.tensor_scalar(zp_sl[:sw, s, :], rz[:sw, M:2 * M],
                                            has_t[:sw, gt:gt + 1], omh_t[:sw, gt:gt + 1],
                                            A.mult, A.add)
                    tile_idx += 1

                # Slab-level GRU blend: out = n + z'*(mem - n)
                d_sl = pool.tile([P, nsub, M], f32, tag="d_sl")
                nc.vector.tensor_tensor(d_sl[:], mem_s[:], n_sl[:], A.subtract)
                f_sl = pool.tile([P, nsub, M], f32, tag="f_sl")
                nc.vector.tensor_tensor(f_sl[:], d_sl[:], zp_sl[:], A.mult)
                out_sl = pool.tile([P, nsub, M], f32, tag="out_sl")
                nc.vector.tensor_tensor(out_sl[:], n_sl[:], f_sl[:], A.add)
                outr_sl = pool.tile([P, nsub, M], f32r, tag="outr_sl")
                nc.vector.tensor_copy(outr_sl[:], out_sl[:])

                # Community partial: commT[M, C] += new_mem_tile.T @ inc_tile
                base = tile_idx - nsub
                for s, (soff, sw) in enumerate(subs):
                    nc.tensor.matmul(comm_acc[:], outr_sl[:sw, s, :], inc_s[:sw, s, :],
                                     start=(base + s == 0), stop=(base + s == NTILES - 1))

                if w == SLAB:
                    nc.sync.dma_start(
                        om[g0:g0 + w].rearrange("(s p) f -> p s f", p=P), out_sl[:])
                else:
                    for s, (soff, sw) in enumerate(subs):
                        nc.sync.dma_start(om[g0 + soff:g0 + soff + sw], out_sl[:sw, s, :])

            cm = pool.tile([M, C], f32)
            nc.scalar.activation(cm[:], comm_acc[:], AF.Copy)
            nc.sync.dma_start(ocm[:], cm[:])

    nc.compile()
    return nc


def _get_program():
    global _COMPILED
    if _COMPILED is None:
        _COMPILED = _build_program()
    return _COMPILED


def kernel(src, dst, t, last_update, event_feat, src_embeds, dst_embeds,
           nodes_memory, incidence, w_time, b_time, W_ih, W_hh, b_ih, b_hh):
    src = np.asarray(src); dst = np.asarray(dst); t = np.asarray(t)
    last_update = np.asarray(last_update)
    event_feat = np.asarray(event_feat, np.float32)
    src_embeds = np.asarray(src_embeds, np.float32)
    dst_embeds = np.asarray(dst_embeds, np.float32)
    nodes_memory = np.asarray(nodes_memory, np.float32)
    incidence = np.asarray(incidence, np.float32)
    w_time = np.asarray(w_time, np.float32); b_time = np.asarray(b_time, np.float32)
    W_ih = np.asarray(W_ih, np.float32); W_hh = np.asarray(W_hh, np.float32)
    b_ih = np.asarray(b_ih, np.float32); b_hh = np.asarray(b_hh, np.float32)

    # ---- Host routing: 'last' aggregation = stable-sort scatter (index-only) ----
    src_all = np.concatenate([src, dst])
    t_all = np.concatenate([t, t])
    perm = np.argsort(t_all, kind="stable")
    win = np.zeros(N, np.int64)
    win[src_all[perm]] = perm          # ascending rank; last write = newest event
    has = np.bincount(src_all, minlength=N) > 0

    dt_ev = t_all - last_update[src_all]      # int32, per event
    dtw = dt_ev[win].astype(np.float32)       # [N]

    # cos arg, fp32 two-step (matches reference rounding), then f64 range
    # reduction to the Sin-LUT domain: cos(x) = sin(x + pi/2 mod 2pi)
    x = dtw[:, None] * w_time[None, :] + b_time[None, :]
    z = x.astype(np.float64) + (np.pi / 2)
    yp = (z - (2 * np.pi) * np.round(z / (2 * np.pi))).astype(np.float32)  # [N, T]

    # Winner event rows (flipped copies share the original arrays)
    lt = win < E
    ge = ~lt
    w0 = np.where(lt, win, win - E)
    emb_s = np.empty((N, D), np.float32)
    emb_d = np.empty((N, D), np.float32)
    emb_s[lt] = src_embeds[w0[lt]]
    emb_s[ge] = dst_embeds[w0[ge]]
    emb_d[lt] = dst_embeds[w0[lt]]
    emb_d[ge] = src_embeds[w0[ge]]
    feat = event_feat[w0]

    has_f = has.astype(np.float32)

    # Replicated params
    bias_row = (b_ih + b_hh).astype(np.float32)[None, :]
    Wt_ext = np.ascontiguousarray(np.concatenate([W_ih[2 * D + F:], bias_row], axis=0))
    Wa_v = np.ascontiguousarray(W_ih[0:D])
    Wb_v = np.ascontiguousarray(W_ih[D:2 * D])
    Wc_v = np.ascontiguousarray(W_ih[2 * D:2 * D + F])
    Whh_v = np.ascontiguousarray(W_hh)

    pad = NTILES * P - NPC
    in_maps = []
    for c in range(NCORES):
        sl = slice(c * NPC, (c + 1) * NPC)
        yT_c = np.empty((T + 1, NPC), np.float32)
        yT_c[0:T] = yp[sl].T
        yT_c[T] = np.float32(np.pi / 2)      # sin -> 1.0: bias lane
        hp = np.concatenate([has_f[sl], np.zeros(pad, np.float32)])
        hp = np.ascontiguousarray(hp.reshape(NTILES, P).T)
        in_maps.append(dict(
            sT=np.ascontiguousarray(emb_s[sl].T),
            dT=np.ascontiguousarray(emb_d[sl].T),
            fT=np.ascontiguousarray(feat[sl].T),
            mT=np.ascontiguousarray(nodes_memory[sl].T),
            yT=yT_c,
            mem=np.ascontiguousarray(nodes_memory[sl]),
            inc=np.ascontiguousarray(incidence[sl]),
            hasp=hp,
            omhp=np.ascontiguousarray(1.0 - hp),
            Wa=Wa_v, Wb=Wb_v, Wc=Wc_v, Wt=Wt_ext, Whh=Whh_v,
        ))

    nc = _get_program()
    res = run_bass_kernel_spmd(nc, in_maps, core_ids=list(range(NCORES)))

    out = np.empty((N + C, M), np.float32)
    comm = np.zeros((M, C), np.float64)
    for c in range(NCORES):
        out[c * NPC:(c + 1) * NPC] = res.results[c]["om"]
        comm += res.results[c]["ocm"]
    out[N:] = comm.T.astype(np.float32)
    return out
